# revision 40
# baseline (speedup 1.0000x reference)
"""Self-contained Trainium2 Bass kernel for nn_MoEWithDeepEP (8 NeuronCores).

Expert-parallel MoE, two launches:
  K1 (data-parallel, 1024 tokens/core): router logits via bf16 hi/lo matmul
     (fp32-exact) + shared-expert SwiGLU on the hi part.  Outputs transposed
     logits and shared-expert output.
  host: sigmoid/top-2/normalize (fp32, mirrors reference), all-to-all token
     dispatch into per-expert capacity buffers, weight packing.
  K2 (expert-parallel, 8 experts/core): grouped SwiGLU GEMMs in f16 on
     host-pre-gathered contiguous buffers.  One packed 1.25MB DMA per expert.
  host: gather/scatter-add combine weighted by routing scores.
"""
import sys
for _p in ("/opt/trn_rl_repo", "/root/.axon_site/_ro/trn_rl_repo"):
    if _p not in sys.path:
        sys.path.insert(0, _p)

import numpy as np

N = 8192          # tokens
D = 512           # model dim
E = 64            # experts
K = 2             # top-k
H = 256           # expert hidden
HS = 512          # shared hidden (H * NSH)
NCORES = 8
E_LOC = E // NCORES   # 8 experts per core
CAP = 512             # static per-expert slot capacity (max observed load 390)
NL = N // NCORES      # 1024 tokens per core (data-parallel dims of k1)
ROUTE_SCALE = 2.5
# k2 packed per-expert blob layout (cols, f16), split in two DMA chunks:
#   chunk A: xdisp | w1   chunk B: w3 | w2
W1C, W3C, W2C, XEC = 4 * H, 4 * H, 2 * D, 4 * CAP
EBA = XEC + W1C               # 3072 cols
EBB = W3C + W2C               # 2048 cols
EBC = EBA + EBB               # 5120 cols


def _mk_bacc():
    from concourse import bacc

    return bacc.Bacc(
        "TRN2",
        target_bir_lowering=False,
        debug=False,
        enable_asserts=False,
        num_devices=NCORES,
    )


def build_kernel1():
    """Router logits (bf16 hi/lo, fp32-exact) + shared expert SwiGLU."""
    import concourse.bass as bass
    import concourse.tile as tile
    from concourse import mybir

    dt = mybir.dt
    AF = mybir.ActivationFunctionType
    OP = mybir.AluOpType
    nc = _mk_bacc()

    # all inputs pre-packed on host to direct [128, cols] SBUF layout;
    # x split per 512-token group so compute can start after the first chunk
    xh_in = nc.dram_tensor("xh_in", [2, 128, 4 * 512], dt.bfloat16,
                           kind="ExternalInput")
    gw_in = nc.dram_tensor("gw_in", [128, 4 * 2 * E], dt.bfloat16, kind="ExternalInput")
    sw1_in = nc.dram_tensor("sw1_in", [128, 4 * HS], dt.bfloat16, kind="ExternalInput")
    sw3_in = nc.dram_tensor("sw3_in", [128, 4 * HS], dt.bfloat16, kind="ExternalInput")
    sw2_in = nc.dram_tensor("sw2_in", [128, 4 * D], dt.bfloat16, kind="ExternalInput")

    lg_out = nc.dram_tensor("lg_out", [64, 2, 512], dt.float32, kind="ExternalOutput")
    ysh_out = nc.dram_tensor("ysh_out", [2, 128, 4, D], dt.bfloat16,
                             kind="ExternalOutput")

    with tile.TileContext(nc) as tc:
        with (
            tc.tile_pool(name="const", bufs=1) as cpool,
            tc.tile_pool(name="rps", bufs=2, space="PSUM") as rpsum,
            tc.tile_pool(name="hps", bufs=4, space="PSUM") as hpsum,
            tc.tile_pool(name="yps", bufs=2, space="PSUM") as ypsum,
            tc.tile_pool(name="work", bufs=2) as wpool,
            tc.tile_pool(name="res", bufs=1) as respool,
        ):
            # input DMA enqueues spread across idle engine queues so the
            # serial ~660ns enqueue cost parallelizes; critical-path tensors
            # (gw, x group 0) keep their own queue in need-order.
            gw = cpool.tile([128, 4, 2 * E], dt.bfloat16)
            nc.gpsimd.dma_start(gw[:], gw_in.ap())
            xh = [cpool.tile([128, 4, 512], dt.bfloat16, tag=f"xh{g}",
                             name=f"xh{g}") for g in range(2)]
            nc.gpsimd.dma_start(xh[0][:], xh_in.ap()[0])
            sw1 = cpool.tile([128, 4, HS], dt.bfloat16)
            nc.gpsimd.dma_start(sw1[:], sw1_in.ap())
            sw3 = cpool.tile([128, 4, HS], dt.bfloat16)
            nc.gpsimd.dma_start(sw3[:], sw3_in.ap())
            nc.gpsimd.dma_start(xh[1][:], xh_in.ap()[1])
            sw2 = cpool.tile([128, 4, D], dt.bfloat16)
            nc.gpsimd.dma_start(sw2[:], sw2_in.ap())
            dummy = cpool.tile([128, 128], dt.bfloat16)
            nc.vector.memset(dummy[:], 0.0)

            # warmup matmuls on the memset tile: keep the PE continuously busy
            # through the input-DMA ramp so real matmuls start at full p-state
            wps = ypsum.tile([64, 128], dt.float32, tag="py")
            for _ in range(40):
                nc.tensor.matmul(wps[:], lhsT=dummy[:, 0:64], rhs=dummy[:, :],
                                 start=True, stop=True)

            lg_sb = respool.tile([64, 2, 512], dt.float32)
            hshs = []

            def router(g):
                # logitsT = ghT@xh (bf16): ~1e-3 logit noise.  The host
                # recomputes exact fp32 logits for the rare tokens whose
                # top2/3 score gap falls inside that noise band, so top-2
                # selection stays exactly reference-equal.
                ps = rpsum.tile([64, 512], dt.float32, tag="lg")
                for c in range(4):
                    nc.tensor.matmul(ps[:], lhsT=gw[:, c, 0:E], rhs=xh[g][:, c, :],
                                     start=(c == 0), stop=(c == 3))
                nc.vector.tensor_copy(lg_sb[:, g, :], ps[:])

            def shared_h(g):
                hsh = wpool.tile([128, 4, 512], dt.bfloat16, tag=f"hsh{g}",
                                 name=f"hsh{g}")
                hshs.append(hsh)
                for hc in range(4):
                    hs = slice(hc * 128, (hc + 1) * 128)
                    ph1 = hpsum.tile([128, 512], dt.float32, tag="ph")
                    for c in range(4):
                        nc.tensor.matmul(ph1[:], lhsT=sw1[:, c, hs],
                                         rhs=xh[g][:, c, :],
                                         start=(c == 0), stop=(c == 3))
                    ph3 = hpsum.tile([128, 512], dt.float32, tag="ph")
                    for c in range(4):
                        nc.tensor.matmul(ph3[:], lhsT=sw3[:, c, hs],
                                         rhs=xh[g][:, c, :],
                                         start=(c == 0), stop=(c == 3))
                    t1 = wpool.tile([128, 512], dt.float32, tag="silu")
                    nc.scalar.activation(t1[:], ph1[:], AF.Silu)
                    nc.vector.tensor_tensor(out=hsh[:, hc, :], in0=t1[:], in1=ph3[:],
                                            op=OP.mult)

            def shared_y(g, stream=False):
                hsh = hshs[g]
                ysh = wpool.tile([128, 4, D], dt.bfloat16, tag="ysh")
                for tc_ in range(4):
                    py = ypsum.tile([128, D], dt.float32, tag="py")
                    for hc in range(4):
                        nc.tensor.matmul(
                            py[:], lhsT=hsh[:, hc, bass.ts(tc_, 128)],
                            rhs=sw2[:, hc, :], start=(hc == 0), stop=(hc == 3),
                        )
                    nc.vector.tensor_copy(ysh[:, tc_, :], py[:])
                    if stream:
                        nc.sync.dma_start(ysh_out.ap()[g][:, tc_, :],
                                          ysh[:, tc_, :])
                if not stream:
                    nc.sync.dma_start(ysh_out.ap()[g], ysh[:])

            router(0)
            shared_h(0)
            router(1)
            nc.sync.dma_start(lg_out.ap(), lg_sb[:])
            shared_h(1)
            shared_y(0)
            shared_y(1)

    nc.compile()
    return nc


def build_kernel2(cnts):
    """Grouped expert SwiGLU GEMMs (f16), specialized to per-slot token counts.

    cnts: tuple of E_LOC ints (ascending), each a multiple of 16 and <= 512.
    Slot j processes cnts[j] dispatch slots; every core runs the same stream,
    with experts assigned to (core, slot) on the host so that slot j's count
    bounds all cores' experts in that slot.
    """
    import concourse.bass as bass
    import concourse.tile as tile
    from concourse import mybir

    dt = mybir.dt
    AF = mybir.ActivationFunctionType
    OP = mybir.AluOpType
    nc = _mk_bacc()

    offs = [0]
    for cnt in cnts:
        offs.append(offs[-1] + (4 * cnt + W1C + EBB))
    tot_in = offs[-1]
    yoffs = [0]
    for cnt in cnts:
        yoffs.append(yoffs[-1] + 4 * cnt)
    tot_out = yoffs[-1]

    ebl = nc.dram_tensor("ebl", [128, tot_in], dt.float16, kind="ExternalInput")
    y_out = nc.dram_tensor("y_out", [128, tot_out], dt.float16,
                           kind="ExternalOutput")

    with tile.TileContext(nc) as tc:
        with (
            tc.tile_pool(name="hps", bufs=4, space="PSUM") as hpsum,
            tc.tile_pool(name="yps", bufs=4, space="PSUM") as ypsum,
            tc.tile_pool(name="sb", bufs=1) as pool,
        ):
            ebAs, ebBs, hes = [], [], []
            dummy = pool.tile([128, 128], dt.float16)
            nc.vector.memset(dummy[:], 0.0)
            # all input DMAs enqueue up front: the Sync queue is serial, and an
            # output DMA waiting on compute must never block the next slot's
            # input transfer (head-of-line blocking stalls the tensor engine)
            for j in range(len(cnts)):
                eba = 4 * cnts[j] + W1C
                ebA = pool.tile([128, eba], dt.float16, name=f"ebA{j}")
                nc.gpsimd.dma_start(ebA[:], ebl.ap()[:, offs[j]:offs[j] + eba])
                ebB = pool.tile([128, EBB], dt.float16, name=f"ebB{j}")
                nc.gpsimd.dma_start(ebB[:],
                                  ebl.ap()[:, offs[j] + eba:offs[j] + eba + EBB])
                ebAs.append(ebA)
                ebBs.append(ebB)

            def h_stage(j):
                cnt = cnts[j]
                xec = 4 * cnt
                ebA, ebB = ebAs[j], ebBs[j]
                he = pool.tile([128, 2, cnt], dt.float16, name=f"he{j}")
                hes.append(he)
                for hc in range(2):
                    ph1 = hpsum.tile([128, cnt], dt.float32, tag="ph")
                    for c in range(4):
                        nc.tensor.matmul(
                            ph1[:], lhsT=ebA[:, xec + c * H + hc * 128:
                                             xec + c * H + (hc + 1) * 128],
                            rhs=ebA[:, c * cnt:(c + 1) * cnt],
                            start=(c == 0), stop=(c == 3))
                    ph3 = hpsum.tile([128, cnt], dt.float32, tag="ph")
                    for c in range(4):
                        nc.tensor.matmul(
                            ph3[:], lhsT=ebB[:, c * H + hc * 128:
                                             c * H + (hc + 1) * 128],
                            rhs=ebA[:, c * cnt:(c + 1) * cnt],
                            start=(c == 0), stop=(c == 3))
                    t1 = pool.tile([128, cnt], dt.float32, name=f"t1_{j}_{hc}")
                    nc.scalar.activation(t1[:], ph1[:], AF.Silu)
                    nc.vector.tensor_tensor(out=he[:, hc, :], in0=t1[:], in1=ph3[:],
                                            op=OP.mult)

            def y_stage(j, stream=False):
                cnt = cnts[j]
                he, ebB = hes[j], ebBs[j]
                # transposed: out yT [D-slice 128, cnt] so rows scale with cnt
                yb = pool.tile([128, 4, cnt], dt.float16, name=f"yb{j}")
                dst = y_out.ap()[:, yoffs[j]:yoffs[j + 1]].rearrange(
                    "p (d t) -> p d t", d=4)
                for ds in range(4):
                    py = ypsum.tile([128, cnt], dt.float32, tag="py")
                    for hc in range(2):
                        nc.tensor.matmul(
                            py[:],
                            lhsT=ebB[:, W3C + hc * D + ds * 128:
                                     W3C + hc * D + (ds + 1) * 128],
                            rhs=he[:, hc, :],
                            start=(hc == 0), stop=(hc == 1),
                        )
                    nc.vector.tensor_copy(yb[:, ds, :], py[:])
                    if stream:
                        nc.sync.dma_start(dst[:, ds, :], yb[:, ds, :])
                if not stream:
                    nc.sync.dma_start(dst, yb[:])

            # warmup matmuls on the memset tile: keep the PE continuously busy
            # through the first blob's DMA so real matmuls start at full p-state
            wps = ypsum.tile([64, 128], dt.float32, tag="py")
            for _ in range(40):
                nc.tensor.matmul(wps[:], lhsT=dummy[:, 0:64], rhs=dummy[:, :],
                                 start=True, stop=True)

            # software pipeline: y-stage runs one slot behind the h-stage so
            # the tensor engine never waits on the silu/mul chain.
            n_slots = len(cnts)
            for j in range(n_slots):
                h_stage(j)
                if j >= 1:
                    y_stage(j - 1)
            y_stage(n_slots - 1)

    nc.compile()
    return nc


# ---------------- host-side sharding / packing / combine ----------------

def _to_pct(a, parts=4):
    """[R, T] (R = parts*128 rows) -> [128, parts*T] with cols (c, t)."""
    r, t = a.shape
    return np.ascontiguousarray(
        a.reshape(parts, 128, t).transpose(1, 0, 2).reshape(128, parts * t)
    )


def host_prepare1(x, gate_w, sw1, sw3, sw2):
    import ml_dtypes

    bf16 = ml_dtypes.bfloat16
    xf = np.asarray(x, dtype=np.float32).reshape(N, D)
    gwT = np.asarray(gate_w, np.float32).T            # [D, E]
    gh = gwT.astype(bf16)
    gl = (gwT - gh.astype(np.float32)).astype(bf16)
    gw_in = _to_pct(np.concatenate([gh, gl], axis=1))
    sw1_in = _to_pct(np.asarray(sw1, np.float32).astype(bf16))
    sw3_in = _to_pct(np.asarray(sw3, np.float32).astype(bf16))
    sw2_in = _to_pct(np.asarray(sw2, np.float32).astype(bf16))
    in_maps = []
    for c in range(NCORES):
        xT = xf[c * NL:(c + 1) * NL].T                # [D, NL]
        xh = xT.astype(bf16)
        # [2 groups, 128, 4*512]
        xh_in = np.stack([_to_pct(xh[:, g * 512:(g + 1) * 512]) for g in range(2)])
        in_maps.append({
            "xh_in": xh_in,
            "gw_in": gw_in,
            "sw1_in": sw1_in,
            "sw3_in": sw3_in,
            "sw2_in": sw2_in,
        })
    return in_maps


def host_route(res1, x, gate_w):
    """fp32 routing from device logits, mirroring the reference.

    Device logits carry ~1e-3 bf16 rounding noise; for tokens whose top2/3
    score gap is inside that band, recompute exact fp32 logits so top-2
    selection matches the fp32 reference exactly.
    """
    logits = np.concatenate(
        [res["lg_out"].reshape(64, NL).T.astype(np.float32) for res in res1], axis=0
    )                                                  # [N, E]
    scores = (1.0 / (1.0 + np.exp(-logits))).astype(np.float32)
    srt = np.sort(scores, axis=1)
    amb = (srt[:, -2] - srt[:, -3]) < np.float32(5e-3)
    if amb.any():
        xf = np.asarray(x, np.float32).reshape(N, D)
        exact = xf[amb] @ np.asarray(gate_w, np.float32).T
        scores[amb] = (1.0 / (1.0 + np.exp(-exact))).astype(np.float32)
    top_idx = np.argsort(-scores, axis=1, kind="stable")[:, :K]   # [N, K]
    s = np.take_along_axis(scores, top_idx, axis=1).astype(np.float32)
    gat = s / (s.sum(1, keepdims=True) + np.float32(1e-20)) * np.float32(ROUTE_SCALE)
    return top_idx, gat.astype(np.float32)


def host_plan2(top_idx):
    """Assign expert token-chunks to (core, slot); static per-slot counts.

    Experts with more than 512 routed tokens are split into multiple <=512
    chunks (extra slots), so any routing distribution is supported.  Slot
    sequence: smallest group first (short DMA ramp), then descending so the
    pipeline drains on the cheapest slot.
    """
    flat_e = top_idx.reshape(-1)                       # [N*K] pair -> expert
    order = np.argsort(flat_e, kind="stable")
    counts = np.bincount(flat_e, minlength=E)
    starts = np.concatenate([[0], np.cumsum(counts)])
    entries = []                                       # (expert, pair indices)
    for e in range(E):
        pairs = order[starts[e]:starts[e + 1]]
        if len(pairs) == 0:
            entries.append((e, pairs))
            continue
        for o in range(0, len(pairs), 512):
            entries.append((e, pairs[o:o + 512]))
    entries.sort(key=lambda t: -len(t[1]))
    n_slots = -(-len(entries) // NCORES)
    entries += [(0, np.zeros(0, np.int64))] * (n_slots * NCORES - len(entries))
    groups = [entries[NCORES * g:NCORES * (g + 1)] for g in range(n_slots)]
    seq = [groups[-1]] + groups[:-1]
    cnts, assign = [], []
    for grp in seq:
        cnt = max(len(p) for _, p in grp)
        cnts.append(min(512, max(16, -(-cnt // 16) * 16)))
        assign.append(grp)
    return assign, tuple(cnts)


def host_prepare2(x, w1, w3, w2, gat, plan):
    assign, cnts = plan
    n_slots = len(cnts)
    xf16 = np.asarray(x, np.float32).reshape(N, D).astype(np.float16)
    w1h = np.asarray(w1, np.float32).astype(np.float16)
    w3h = np.asarray(w3, np.float32).astype(np.float16)
    w2h = np.asarray(w2, np.float32).astype(np.float16)
    gflat = gat.reshape(-1)
    tot_in = sum(4 * cnt + W1C + EBB for cnt in cnts)

    wpct = {}

    def wblk(e):
        if e not in wpct:
            wpct[e] = (_to_pct(w1h[e]), _to_pct(w3h[e]), _to_pct(w2h[e], parts=2))
        return wpct[e]

    in_maps, comb = [], []
    for c in range(NCORES):
        ebl = np.zeros((128, tot_in), np.float16)
        cinfo = []
        o = 0
        for j in range(n_slots):
            cnt = cnts[j]
            e, pairs = assign[j][c]
            toks = pairs // K
            n = len(toks)
            xd = np.zeros((cnt, D), np.float16)
            xd[:n] = xf16[toks]
            ebl[:, o:o + 4 * cnt] = _to_pct(np.ascontiguousarray(xd.T))
            o += 4 * cnt
            b1, b3, b2 = wblk(int(e))
            ebl[:, o:o + W1C] = b1
            ebl[:, o + W1C:o + W1C + W3C] = b3
            ebl[:, o + W1C + W3C:o + EBB + W1C] = b2
            o += W1C + EBB
            cinfo.append((toks, gflat[pairs].astype(np.float32)))
        in_maps.append({"ebl": ebl})
        comb.append(cinfo)
    return in_maps, comb


def host_combine(res1, res2, comb, cnts):
    out = np.zeros((N, D), dtype=np.float32)
    for c in range(NCORES):
        y = res2[c]["y_out"]                           # [128, tot_out] f16
        o = 0
        for j in range(len(cnts)):
            cnt = cnts[j]
            toks, g = comb[c][j]
            n = len(toks)
            arr = y[:, o:o + 4 * cnt].reshape(128, 4, cnt)
            o += 4 * cnt
            if n:
                yrows = (arr.transpose(1, 0, 2).reshape(D, cnt).T)[:n].astype(
                    np.float32)
                np.add.at(out, toks, yrows * g[:, None])
        ysh = res1[c]["ysh_out"].astype(np.float32)    # [2, 128, 4, D]
        out[c * NL:(c + 1) * NL] += ysh.transpose(0, 2, 1, 3).reshape(NL, D)
    return out.reshape(4, 2048, D)


_CACHE = {}


def kernel(x, gate_w, w1, w3, w2, sw1, sw3, sw2):
    from concourse.bass_utils import run_bass_kernel_spmd

    if "nc1" not in _CACHE:
        _CACHE["nc1"] = build_kernel1()
    nc1 = _CACHE["nc1"]

    def runner(nc, in_maps):
        return run_bass_kernel_spmd(
            nc, in_maps, core_ids=list(range(NCORES))
        ).results

    in1 = host_prepare1(x, gate_w, sw1, sw3, sw2)
    res1 = runner(nc1, in1)
    top_idx, gat = host_route(res1, x, gate_w)
    plan = host_plan2(top_idx)
    cnts = plan[1]
    if ("nc2", cnts) not in _CACHE:
        _CACHE[("nc2", cnts)] = build_kernel2(cnts)
    nc2 = _CACHE[("nc2", cnts)]
    in2, comb = host_prepare2(x, w1, w3, w2, gat, plan)
    res2 = runner(nc2, in2)
    return host_combine(res1, res2, comb, cnts).astype(np.float32)


# revision 41
# speedup vs baseline: 1.0643x; 1.0643x over previous
"""Self-contained Trainium2 Bass kernel for nn_MoEWithDeepEP (8 NeuronCores).

Expert-parallel MoE, two launches:
  K1 (data-parallel, 1024 tokens/core): router logits via bf16 hi/lo matmul
     (fp32-exact) + shared-expert SwiGLU on the hi part.  Outputs transposed
     logits and shared-expert output.
  host: sigmoid/top-2/normalize (fp32, mirrors reference), all-to-all token
     dispatch into per-expert capacity buffers, weight packing.
  K2 (expert-parallel, 8 experts/core): grouped SwiGLU GEMMs in f16 on
     host-pre-gathered contiguous buffers.  One packed 1.25MB DMA per expert.
  host: gather/scatter-add combine weighted by routing scores.
"""
import sys
for _p in ("/opt/trn_rl_repo", "/root/.axon_site/_ro/trn_rl_repo"):
    if _p not in sys.path:
        sys.path.insert(0, _p)

import numpy as np

N = 8192          # tokens
D = 512           # model dim
E = 64            # experts
K = 2             # top-k
H = 256           # expert hidden
HS = 512          # shared hidden (H * NSH)
NCORES = 8
E_LOC = E // NCORES   # 8 experts per core
CAP = 512             # static per-expert slot capacity (max observed load 390)
NL = N // NCORES      # 1024 tokens per core (data-parallel dims of k1)
ROUTE_SCALE = 2.5
# k2 packed per-expert blob layout (cols, f16), split in two DMA chunks:
#   chunk A: xdisp | w1   chunk B: w3 | w2
W1C, W3C, W2C, XEC = 4 * H, 4 * H, 2 * D, 4 * CAP
EBA = XEC + W1C               # 3072 cols
EBB = W3C + W2C               # 2048 cols
EBC = EBA + EBB               # 5120 cols


def _mk_bacc():
    from concourse import bacc

    return bacc.Bacc(
        "TRN2",
        target_bir_lowering=False,
        debug=False,
        enable_asserts=False,
        num_devices=NCORES,
    )


def build_kernel1():
    """Router logits (bf16 hi/lo, fp32-exact) + shared expert SwiGLU."""
    import concourse.bass as bass
    import concourse.tile as tile
    from concourse import mybir

    dt = mybir.dt
    AF = mybir.ActivationFunctionType
    OP = mybir.AluOpType
    nc = _mk_bacc()

    # all inputs pre-packed on host to direct [128, cols] SBUF layout;
    # x split per 512-token group so compute can start after the first chunk
    xh_in = nc.dram_tensor("xh_in", [2, 128, 4 * 512], dt.bfloat16,
                           kind="ExternalInput")
    gw_in = nc.dram_tensor("gw_in", [128, 4 * 2 * E], dt.bfloat16, kind="ExternalInput")
    sw1_in = nc.dram_tensor("sw1_in", [128, 4 * HS], dt.bfloat16, kind="ExternalInput")
    sw3_in = nc.dram_tensor("sw3_in", [128, 4 * HS], dt.bfloat16, kind="ExternalInput")
    sw2_in = nc.dram_tensor("sw2_in", [128, 4 * D], dt.bfloat16, kind="ExternalInput")

    lg_out = nc.dram_tensor("lg_out", [64, 2, 512], dt.float32, kind="ExternalOutput")
    ysh_out = nc.dram_tensor("ysh_out", [2, 128, 4, D], dt.bfloat16,
                             kind="ExternalOutput")

    with tile.TileContext(nc) as tc:
        with (
            tc.tile_pool(name="const", bufs=1) as cpool,
            tc.tile_pool(name="rps", bufs=2, space="PSUM") as rpsum,
            tc.tile_pool(name="hps", bufs=4, space="PSUM") as hpsum,
            tc.tile_pool(name="yps", bufs=2, space="PSUM") as ypsum,
            tc.tile_pool(name="work", bufs=2) as wpool,
            tc.tile_pool(name="res", bufs=1) as respool,
        ):
            # input DMA enqueues spread across idle engine queues so the
            # serial ~660ns enqueue cost parallelizes; critical-path tensors
            # (gw, x group 0) keep their own queue in need-order.
            gw = cpool.tile([128, 4, 2 * E], dt.bfloat16)
            nc.gpsimd.dma_start(gw[:], gw_in.ap())
            xh = [cpool.tile([128, 4, 512], dt.bfloat16, tag=f"xh{g}",
                             name=f"xh{g}") for g in range(2)]
            nc.gpsimd.dma_start(xh[0][:], xh_in.ap()[0])
            sw1 = cpool.tile([128, 4, HS], dt.bfloat16)
            nc.gpsimd.dma_start(sw1[:], sw1_in.ap())
            sw3 = cpool.tile([128, 4, HS], dt.bfloat16)
            nc.gpsimd.dma_start(sw3[:], sw3_in.ap())
            nc.gpsimd.dma_start(xh[1][:], xh_in.ap()[1])
            sw2 = cpool.tile([128, 4, D], dt.bfloat16)
            nc.gpsimd.dma_start(sw2[:], sw2_in.ap())
            dummy = cpool.tile([128, 128], dt.bfloat16)
            nc.vector.memset(dummy[:], 0.0)

            # warmup matmuls on the memset tile: keep the PE continuously busy
            # through the input-DMA ramp so real matmuls start at full p-state
            wps = ypsum.tile([64, 128], dt.float32, tag="py")
            for _ in range(40):
                nc.tensor.matmul(wps[:], lhsT=dummy[:, 0:64], rhs=dummy[:, :],
                                 start=True, stop=True)

            lg_sb = respool.tile([64, 2, 512], dt.float32)
            hshs = []

            def router(g):
                # logitsT = ghT@xh (bf16): ~1e-3 logit noise.  The host
                # recomputes exact fp32 logits for the rare tokens whose
                # top2/3 score gap falls inside that noise band, so top-2
                # selection stays exactly reference-equal.
                ps = rpsum.tile([64, 512], dt.float32, tag="lg")
                for c in range(4):
                    nc.tensor.matmul(ps[:], lhsT=gw[:, c, 0:E], rhs=xh[g][:, c, :],
                                     start=(c == 0), stop=(c == 3))
                nc.vector.tensor_copy(lg_sb[:, g, :], ps[:])

            def shared_h(g):
                hsh = wpool.tile([128, 4, 512], dt.bfloat16, tag=f"hsh{g}",
                                 name=f"hsh{g}")
                hshs.append(hsh)
                for hc in range(4):
                    hs = slice(hc * 128, (hc + 1) * 128)
                    ph1 = hpsum.tile([128, 512], dt.float32, tag="ph")
                    for c in range(4):
                        nc.tensor.matmul(ph1[:], lhsT=sw1[:, c, hs],
                                         rhs=xh[g][:, c, :],
                                         start=(c == 0), stop=(c == 3))
                    ph3 = hpsum.tile([128, 512], dt.float32, tag="ph")
                    for c in range(4):
                        nc.tensor.matmul(ph3[:], lhsT=sw3[:, c, hs],
                                         rhs=xh[g][:, c, :],
                                         start=(c == 0), stop=(c == 3))
                    t1 = wpool.tile([128, 512], dt.float32, tag="silu")
                    nc.scalar.activation(t1[:], ph1[:], AF.Silu)
                    nc.vector.tensor_tensor(out=hsh[:, hc, :], in0=t1[:], in1=ph3[:],
                                            op=OP.mult)

            def shared_y(g, stream=False):
                hsh = hshs[g]
                ysh = wpool.tile([128, 4, D], dt.bfloat16, tag="ysh")
                for tc_ in range(4):
                    py = ypsum.tile([128, D], dt.float32, tag="py")
                    for hc in range(4):
                        nc.tensor.matmul(
                            py[:], lhsT=hsh[:, hc, bass.ts(tc_, 128)],
                            rhs=sw2[:, hc, :], start=(hc == 0), stop=(hc == 3),
                        )
                    nc.vector.tensor_copy(ysh[:, tc_, :], py[:])
                    if stream:
                        nc.sync.dma_start(ysh_out.ap()[g][:, tc_, :],
                                          ysh[:, tc_, :])
                if not stream:
                    nc.sync.dma_start(ysh_out.ap()[g], ysh[:])

            router(0)
            shared_h(0)
            router(1)
            nc.sync.dma_start(lg_out.ap(), lg_sb[:])
            shared_h(1)
            shared_y(0)
            shared_y(1)

    nc.compile()
    return nc


def build_kernel2(cnts):
    """Grouped expert SwiGLU GEMMs (f16), specialized to per-slot token counts.

    cnts: tuple of E_LOC ints (ascending), each a multiple of 16 and <= 512.
    Slot j processes cnts[j] dispatch slots; every core runs the same stream,
    with experts assigned to (core, slot) on the host so that slot j's count
    bounds all cores' experts in that slot.
    """
    import concourse.bass as bass
    import concourse.tile as tile
    from concourse import mybir

    dt = mybir.dt
    AF = mybir.ActivationFunctionType
    OP = mybir.AluOpType
    nc = _mk_bacc()

    offs = [0]
    for cnt in cnts:
        offs.append(offs[-1] + (4 * cnt + W1C + EBB))
    tot_in = offs[-1]
    yoffs = [0]
    for cnt in cnts:
        yoffs.append(yoffs[-1] + 4 * cnt)
    tot_out = yoffs[-1]

    ebl = nc.dram_tensor("ebl", [128, tot_in], dt.float16, kind="ExternalInput")
    y_out = nc.dram_tensor("y_out", [128, tot_out], dt.float16,
                           kind="ExternalOutput")

    with tile.TileContext(nc) as tc:
        with (
            tc.tile_pool(name="hps", bufs=4, space="PSUM") as hpsum,
            tc.tile_pool(name="yps", bufs=4, space="PSUM") as ypsum,
            tc.tile_pool(name="sb", bufs=1) as pool,
        ):
            ebAs, ebBs, hes = [], [], []
            dummy = pool.tile([128, 128], dt.float16)
            nc.vector.memset(dummy[:], 0.0)
            # all input DMAs enqueue up front: the Sync queue is serial, and an
            # output DMA waiting on compute must never block the next slot's
            # input transfer (head-of-line blocking stalls the tensor engine)
            for j in range(len(cnts)):
                eba = 4 * cnts[j] + W1C
                ebA = pool.tile([128, eba], dt.float16, name=f"ebA{j}")
                nc.gpsimd.dma_start(ebA[:], ebl.ap()[:, offs[j]:offs[j] + eba])
                ebB = pool.tile([128, EBB], dt.float16, name=f"ebB{j}")
                nc.gpsimd.dma_start(ebB[:],
                                  ebl.ap()[:, offs[j] + eba:offs[j] + eba + EBB])
                ebAs.append(ebA)
                ebBs.append(ebB)

            def h_stage(j):
                cnt = cnts[j]
                xec = 4 * cnt
                ebA, ebB = ebAs[j], ebBs[j]
                he = pool.tile([128, 2, cnt], dt.float16, name=f"he{j}")
                hes.append(he)
                for hc in range(2):
                    ph1 = hpsum.tile([128, cnt], dt.float32, tag="ph")
                    for c in range(4):
                        nc.tensor.matmul(
                            ph1[:], lhsT=ebA[:, xec + c * H + hc * 128:
                                             xec + c * H + (hc + 1) * 128],
                            rhs=ebA[:, c * cnt:(c + 1) * cnt],
                            start=(c == 0), stop=(c == 3))
                    ph3 = hpsum.tile([128, cnt], dt.float32, tag="ph")
                    for c in range(4):
                        nc.tensor.matmul(
                            ph3[:], lhsT=ebB[:, c * H + hc * 128:
                                             c * H + (hc + 1) * 128],
                            rhs=ebA[:, c * cnt:(c + 1) * cnt],
                            start=(c == 0), stop=(c == 3))
                    t1 = pool.tile([128, cnt], dt.float32, name=f"t1_{j}_{hc}")
                    nc.scalar.activation(t1[:], ph1[:], AF.Silu)
                    nc.vector.tensor_tensor(out=he[:, hc, :], in0=t1[:], in1=ph3[:],
                                            op=OP.mult)

            def y_stage(j, stream=False):
                cnt = cnts[j]
                he, ebB = hes[j], ebBs[j]
                # transposed: out yT [D-slice 128, cnt] so rows scale with cnt
                yb = pool.tile([128, 4, cnt], dt.float16, name=f"yb{j}")
                dst = y_out.ap()[:, yoffs[j]:yoffs[j + 1]].rearrange(
                    "p (d t) -> p d t", d=4)
                for ds in range(4):
                    py = ypsum.tile([128, cnt], dt.float32, tag="py")
                    for hc in range(2):
                        nc.tensor.matmul(
                            py[:],
                            lhsT=ebB[:, W3C + hc * D + ds * 128:
                                     W3C + hc * D + (ds + 1) * 128],
                            rhs=he[:, hc, :],
                            start=(hc == 0), stop=(hc == 1),
                        )
                    nc.vector.tensor_copy(yb[:, ds, :], py[:])
                    if stream:
                        nc.sync.dma_start(dst[:, ds, :], yb[:, ds, :])
                if not stream:
                    nc.sync.dma_start(dst, yb[:])

            # warmup matmuls on the memset tile: keep the PE continuously busy
            # through the first blob's DMA so real matmuls start at full p-state
            wps = ypsum.tile([64, 128], dt.float32, tag="py")
            for _ in range(40):
                nc.tensor.matmul(wps[:], lhsT=dummy[:, 0:64], rhs=dummy[:, :],
                                 start=True, stop=True)

            # software pipeline: y-stage runs two slots behind the h-stage so
            # the silu/mul chain (scalar+vector) has a full h-stage of slack
            # before its y matmuls are reached in the tensor queue.
            n_slots = len(cnts)
            for j in range(n_slots):
                h_stage(j)
                if j >= 2:
                    y_stage(j - 2)
            y_stage(n_slots - 2)
            y_stage(n_slots - 1)

    nc.compile()
    return nc


# ---------------- host-side sharding / packing / combine ----------------

def _to_pct(a, parts=4):
    """[R, T] (R = parts*128 rows) -> [128, parts*T] with cols (c, t)."""
    r, t = a.shape
    return np.ascontiguousarray(
        a.reshape(parts, 128, t).transpose(1, 0, 2).reshape(128, parts * t)
    )


def host_prepare1(x, gate_w, sw1, sw3, sw2):
    import ml_dtypes

    bf16 = ml_dtypes.bfloat16
    xf = np.asarray(x, dtype=np.float32).reshape(N, D)
    gwT = np.asarray(gate_w, np.float32).T            # [D, E]
    gh = gwT.astype(bf16)
    gl = (gwT - gh.astype(np.float32)).astype(bf16)
    gw_in = _to_pct(np.concatenate([gh, gl], axis=1))
    sw1_in = _to_pct(np.asarray(sw1, np.float32).astype(bf16))
    sw3_in = _to_pct(np.asarray(sw3, np.float32).astype(bf16))
    sw2_in = _to_pct(np.asarray(sw2, np.float32).astype(bf16))
    in_maps = []
    for c in range(NCORES):
        xT = xf[c * NL:(c + 1) * NL].T                # [D, NL]
        xh = xT.astype(bf16)
        # [2 groups, 128, 4*512]
        xh_in = np.stack([_to_pct(xh[:, g * 512:(g + 1) * 512]) for g in range(2)])
        in_maps.append({
            "xh_in": xh_in,
            "gw_in": gw_in,
            "sw1_in": sw1_in,
            "sw3_in": sw3_in,
            "sw2_in": sw2_in,
        })
    return in_maps


def host_route(res1, x, gate_w):
    """fp32 routing from device logits, mirroring the reference.

    Device logits carry ~1e-3 bf16 rounding noise; for tokens whose top2/3
    score gap is inside that band, recompute exact fp32 logits so top-2
    selection matches the fp32 reference exactly.
    """
    logits = np.concatenate(
        [res["lg_out"].reshape(64, NL).T.astype(np.float32) for res in res1], axis=0
    )                                                  # [N, E]
    scores = (1.0 / (1.0 + np.exp(-logits))).astype(np.float32)
    srt = np.sort(scores, axis=1)
    amb = (srt[:, -2] - srt[:, -3]) < np.float32(5e-3)
    if amb.any():
        xf = np.asarray(x, np.float32).reshape(N, D)
        exact = xf[amb] @ np.asarray(gate_w, np.float32).T
        scores[amb] = (1.0 / (1.0 + np.exp(-exact))).astype(np.float32)
    top_idx = np.argsort(-scores, axis=1, kind="stable")[:, :K]   # [N, K]
    s = np.take_along_axis(scores, top_idx, axis=1).astype(np.float32)
    gat = s / (s.sum(1, keepdims=True) + np.float32(1e-20)) * np.float32(ROUTE_SCALE)
    return top_idx, gat.astype(np.float32)


def host_plan2(top_idx):
    """Assign expert token-chunks to (core, slot); static per-slot counts.

    Experts with more than 512 routed tokens are split into multiple <=512
    chunks (extra slots), so any routing distribution is supported.  Slot
    sequence: smallest group first (short DMA ramp), then descending so the
    pipeline drains on the cheapest slot.
    """
    flat_e = top_idx.reshape(-1)                       # [N*K] pair -> expert
    order = np.argsort(flat_e, kind="stable")
    counts = np.bincount(flat_e, minlength=E)
    starts = np.concatenate([[0], np.cumsum(counts)])
    entries = []                                       # (expert, pair indices)
    for e in range(E):
        pairs = order[starts[e]:starts[e + 1]]
        if len(pairs) == 0:
            entries.append((e, pairs))
            continue
        for o in range(0, len(pairs), 512):
            entries.append((e, pairs[o:o + 512]))
    entries.sort(key=lambda t: -len(t[1]))
    n_slots = -(-len(entries) // NCORES)
    entries += [(0, np.zeros(0, np.int64))] * (n_slots * NCORES - len(entries))
    groups = [entries[NCORES * g:NCORES * (g + 1)] for g in range(n_slots)]
    seq = [groups[-1]] + groups[:-1]
    cnts, assign = [], []
    for grp in seq:
        cnt = max(len(p) for _, p in grp)
        cnts.append(min(512, max(16, -(-cnt // 16) * 16)))
        assign.append(grp)
    return assign, tuple(cnts)


def host_prepare2(x, w1, w3, w2, gat, plan):
    assign, cnts = plan
    n_slots = len(cnts)
    xf16 = np.asarray(x, np.float32).reshape(N, D).astype(np.float16)
    w1h = np.asarray(w1, np.float32).astype(np.float16)
    w3h = np.asarray(w3, np.float32).astype(np.float16)
    w2h = np.asarray(w2, np.float32).astype(np.float16)
    gflat = gat.reshape(-1)
    tot_in = sum(4 * cnt + W1C + EBB for cnt in cnts)

    wpct = {}

    def wblk(e):
        if e not in wpct:
            wpct[e] = (_to_pct(w1h[e]), _to_pct(w3h[e]), _to_pct(w2h[e], parts=2))
        return wpct[e]

    in_maps, comb = [], []
    for c in range(NCORES):
        ebl = np.zeros((128, tot_in), np.float16)
        cinfo = []
        o = 0
        for j in range(n_slots):
            cnt = cnts[j]
            e, pairs = assign[j][c]
            toks = pairs // K
            n = len(toks)
            xd = np.zeros((cnt, D), np.float16)
            xd[:n] = xf16[toks]
            ebl[:, o:o + 4 * cnt] = _to_pct(np.ascontiguousarray(xd.T))
            o += 4 * cnt
            b1, b3, b2 = wblk(int(e))
            ebl[:, o:o + W1C] = b1
            ebl[:, o + W1C:o + W1C + W3C] = b3
            ebl[:, o + W1C + W3C:o + EBB + W1C] = b2
            o += W1C + EBB
            cinfo.append((toks, gflat[pairs].astype(np.float32)))
        in_maps.append({"ebl": ebl})
        comb.append(cinfo)
    return in_maps, comb


def host_combine(res1, res2, comb, cnts):
    out = np.zeros((N, D), dtype=np.float32)
    for c in range(NCORES):
        y = res2[c]["y_out"]                           # [128, tot_out] f16
        o = 0
        for j in range(len(cnts)):
            cnt = cnts[j]
            toks, g = comb[c][j]
            n = len(toks)
            arr = y[:, o:o + 4 * cnt].reshape(128, 4, cnt)
            o += 4 * cnt
            if n:
                yrows = (arr.transpose(1, 0, 2).reshape(D, cnt).T)[:n].astype(
                    np.float32)
                np.add.at(out, toks, yrows * g[:, None])
        ysh = res1[c]["ysh_out"].astype(np.float32)    # [2, 128, 4, D]
        out[c * NL:(c + 1) * NL] += ysh.transpose(0, 2, 1, 3).reshape(NL, D)
    return out.reshape(4, 2048, D)


_CACHE = {}


def kernel(x, gate_w, w1, w3, w2, sw1, sw3, sw2):
    from concourse.bass_utils import run_bass_kernel_spmd

    if "nc1" not in _CACHE:
        _CACHE["nc1"] = build_kernel1()
    nc1 = _CACHE["nc1"]

    def runner(nc, in_maps):
        return run_bass_kernel_spmd(
            nc, in_maps, core_ids=list(range(NCORES))
        ).results

    in1 = host_prepare1(x, gate_w, sw1, sw3, sw2)
    res1 = runner(nc1, in1)
    top_idx, gat = host_route(res1, x, gate_w)
    plan = host_plan2(top_idx)
    cnts = plan[1]
    if ("nc2", cnts) not in _CACHE:
        _CACHE[("nc2", cnts)] = build_kernel2(cnts)
    nc2 = _CACHE[("nc2", cnts)]
    in2, comb = host_prepare2(x, w1, w3, w2, gat, plan)
    res2 = runner(nc2, in2)
    return host_combine(res1, res2, comb, cnts).astype(np.float32)
